# revision 2
# baseline (speedup 1.0000x reference)
"""SimCLR NT-Xent contrastive loss on 8 Trainium2 cores.

Reference math (B=2048, D=256, T=0.5):
    zn = l2norm_rows(concat(z_i, z_j))          # [4096, 256]
    sim = zn @ zn.T / T                         # [4096, 4096]
    loss = mean_g [ log(sum_j exp(sim[g,j]) - exp(sim[g,g])) - sim[g, (g+B)%N] ]

Sharding: the 4096 sim rows are split across 8 cores (512 rows each).  Each
core receives the FULL z, rolled so that "its" rows sit at positions 0..511
and pre-transposed to [D, N] on the host (pure layout prep — no arithmetic).
With the roll, every core runs the identical static program:
  rows   = columns 0:512   of znT
  pos    = columns 2048:2560 of znT  (the (g+B)%N positive pairs)
Each core emits one fp32 partial: sum_g [log(rowsum_g - e^2) - 2*posdot_g];
the host sums the 8 partials and divides by 4096.  (sim[g,g] == 1/T exactly
for l2-normalized rows, so exp(diag) == e^2 up to fp32 noise ~1e-7.)

Per-core dataflow (all engines in play), pipelined per 512-column chunk in
four pair-groups so normalization streams right behind the input DMA:
  DMA    zt [256, 4096] fp32, column-chunked, interleaved [128, 2, 512]
  DVE    sq = zt*zt (bf16)
  PE     sumsq via M=32 all-ones matmul -> 32 replicated rows per chunk at
         partition 32r of a shared PSUM bank (K=256 over both d-halves)
  ACT    inv = exp(-0.5*ln(ss)) straight off PSUM (rsqrt via the single
         ln+exp table set; ACT Rsqrt is banned for accuracy)
  PE     replicate row 32r to all 128 partitions via a K=1 ones matmul
  DVE    znT = zt * invrep  (bf16)   -> normalized transposed z
  PE     sim block matmuls: lhsT = znT cols [rc*128..], rhs = znT col chunks,
         K=256 in 2 passes, PSUM [128, 1024] aligned with the pair-groups
  ACT    exp(2*sim) with accum_out -> per-row partial rowsums
  DVE    posdot via scalar_tensor_tensor(scale=-2) with accum_out
  ACT    log(rowsum - e^2)
  DVE+GPSIMD  reduce the [128, 6] tail -> [1, 1] partial, DMA out
"""

import numpy as np

import concourse.bacc as bacc
import concourse.bass as bass
import concourse.bass_isa as bass_isa
import concourse.tile as tile
from concourse import mybir

F32 = mybir.dt.float32
BF16 = mybir.dt.bfloat16
AF = mybir.ActivationFunctionType
ALU = mybir.AluOpType
AXIS = mybir.AxisListType

B = 2048
D = 256
N = 2 * B            # 4096 total rows
NCORES = 8
RPC = N // NCORES    # 512 rows per core
CC = 8               # column chunks of 512
CW = N // CC         # 512 chunk width
E2 = float(np.exp(np.float32(2.0)))   # exp(sim[g,g]) = exp(1/T)
IN_NAMES = ("zt",)   # ExternalInput order fed to the bass exec primitive
OUT_SHAPE = (1, 1)   # per-core output shape


class _Bacc(bacc.Bacc):
    """Bacc that pins the activation-table pass to the one set containing
    both Ln and Exp — the default fixpoint picks per-function sets and
    thrashes 5 table loads (~6.4us of ACT) into the schedule."""

    def insert_act_table_loads(self):
        from concourse.hw_specs import get_activation_tables
        import bass_rust as _bass_rust

        has_activation = any(
            isinstance(i, mybir.InstActivation)
            for b in self.main_func.blocks
            for i in b.instructions
        )
        if not has_activation:
            return
        # Keep the full list (act_func_set_id is the index into
        # act_info.json's act_func_sets!) but make the combined set the only
        # candidate for Ln/Exp so the pass can't alternate between
        # single-function sets.
        keep = {
            mybir.ActivationFunctionType.Ln,
            mybir.ActivationFunctionType.Exp,
        }
        tables = [
            (k, v if k == "natural_log_exp_and_others" else v - keep)
            for k, v in get_activation_tables(self.m.arch).items()
        ]
        _bass_rust.insert_act_table_loads(self, tables)


def build_nc():
    nc = _Bacc("TRN2", target_bir_lowering=False, debug=False)
    zt = nc.dram_tensor("zt", [D, N], F32, kind="ExternalInput").ap()
    out = nc.dram_tensor("out", [1, 1], F32, kind="ExternalOutput").ap()
    with tile.TileContext(nc) as tc:
        build_tile_program(tc, out, zt)
    nc.compile()
    return nc


def build_tile_program(tc: tile.TileContext, out: bass.AP, zt: bass.AP):
    nc = tc.nc
    # zt[d, n] viewed as [p, h, n] with d = h*128 + p
    zt_v = zt.rearrange("(h p) n -> p h n", h=2)

    with (
        tc.tile_pool(name="consts", bufs=1) as consts,
        tc.tile_pool(name="ztp", bufs=8) as ztp,
        tc.tile_pool(name="sqp", bufs=4) as sqp,
        tc.tile_pool(name="smalls", bufs=1) as smalls,
        tc.tile_pool(name="zntp", bufs=1) as zntp,
        tc.tile_pool(name="scrp", bufs=2) as scrp,
        tc.tile_pool(name="sspsp", bufs=1, space="PSUM") as sspsp,
        tc.tile_pool(name="invp", bufs=1, space="PSUM") as invp,
        tc.tile_pool(name="simp", bufs=3, space="PSUM") as simp,
    ):
        ones_sq = consts.tile([128, 128], F32, tag="ones_sq")
        nc.vector.memset(ones_sq, 1.0)
        ones_col32 = consts.tile([128, 32], BF16, tag="ones_col32")
        nc.vector.memset(ones_col32, 1.0)
        zero_col = consts.tile([128, 1], F32, tag="zero_col")
        nc.vector.memset(zero_col, 0.0)
        neg_e2 = consts.tile([128, 1], F32, tag="neg_e2")
        nc.vector.memset(neg_e2, -E2)

        # persistent small tiles
        znt = zntp.tile([128, 2, N], BF16)
        acc16 = smalls.tile([128, 16], F32, tag="acc16")
        tail6 = smalls.tile([128, 6], F32, tag="tail6")
        rowsum4 = smalls.tile([128, 4], F32, tag="rowsum4")
        tail1 = smalls.tile([128, 1], F32, tag="tail1")
        result = smalls.tile([128, 1], F32, tag="result")

        # ---- normalization pipeline.  Column chunks of 512 in four groups
        # of 2, so normalization streams right behind the input DMA and the
        # first sim matmuls (and ACT exps) start early.  Per chunk: sumsq
        # via an M=32 all-ones matmul (32 identical rows at partition 32r)
        # accumulating both d-halves in one PSUM bank.  Per group: rsqrt
        # straight off PSUM via exp(-0.5*ln); per chunk: K=1 matmul
        # replicates row 32r to all 128 partitions, then the normalize
        # multiplies produce bf16 znT.
        zt_tiles = {}
        sq_tiles = {}

        def load_and_sq(cc):
            ztc = ztp.tile([128, 2, CW], F32, tag="ztc", name=f"ztc{cc}")
            sqc = sqp.tile([128, 2, CW], BF16, tag="sqc", name=f"sqc{cc}")
            # stream each chunk in 256-column halves so square + sumsq
            # pipeline inside the DMA window instead of trailing it
            for q in range(2):
                qs = slice(q * 256, (q + 1) * 256)
                gqs = slice(cc * CW + q * 256, cc * CW + (q + 1) * 256)
                nc.sync.dma_start(out=ztc[:, :, qs], in_=zt_v[:, :, gqs])
                # split the square across DVE/GPSIMD so DVE (which also owns
                # the normalize multiplies) isn't the group-cadence governor
                nc.vector.tensor_mul(sqc[:, 0, qs], ztc[:, 0, qs], ztc[:, 0, qs])
                nc.gpsimd.tensor_mul(sqc[:, 1, qs], ztc[:, 1, qs], ztc[:, 1, qs])
            zt_tiles[cc] = ztc
            sq_tiles[cc] = sqc

        # software-pipelined emission: group g+1's load+sq are emitted (and
        # so FIFO-ordered) ahead of group g's dependent norm ops, avoiding
        # head-of-line stalls on the DVE queue
        load_and_sq(0)
        load_and_sq(1)
        for g in range(4):
            for r in range(2):
                nxt = 2 * (g + 1) + r
                if nxt < CC:
                    load_and_sq(nxt)
            ssps = sspsp.tile([128, CW], F32, tag="ssps")
            for r in range(2):
                cc = 2 * g + r
                for q in range(2):
                    qs = slice(q * 256, (q + 1) * 256)
                    for h in range(2):
                        nc.tensor.matmul(
                            ssps[32 * r : 32 * r + 32, qs],
                            ones_col32,
                            sq_tiles[cc][:, h, qs],
                            start=(h == 0),
                            stop=(h == 1),
                            tile_position=(0, 32 * r),
                        )

            # inv = exp(-0.5 * ln(sumsq)) = 1/sqrt(sumsq)  (ACT reads PSUM)
            lng = smalls.tile([64, CW], F32, tag="lng", bufs=2)
            invg = smalls.tile([64, CW], F32, tag="invg", bufs=2)
            nc.scalar.activation(lng, ssps[0:64, :], AF.Ln, bias=zero_col[0:64, :])
            nc.scalar.activation(
                invg, lng, AF.Exp, bias=zero_col[0:64, :], scale=-0.5
            )

            for r in range(2):
                cc = 2 * g + r
                cols = slice(cc * CW, (cc + 1) * CW)
                # replicate chunk cc's inv row (partition 32r) across all 128
                # partitions: K=1 outer product; lhsT/rhs share base 32r and
                # tile_position auto-derives to (32r, 0)
                invrep = invp.tile([128, CW], F32, tag="invrep")
                nc.tensor.matmul(
                    invrep,
                    ones_sq[32 * r : 32 * r + 1, :],
                    invg[32 * r : 32 * r + 1, :],
                    start=True,
                    stop=True,
                )
                # znT chunk = zt * invrep  (bf16), both d-halves
                for h in range(2):
                    nc.vector.tensor_mul(
                        znt[:, h, cols], zt_tiles[cc][:, h, :], invrep
                    )

        # ---- positive-pair dots: rows 0:512 vs rows 2048:2560
        # tail6[:, 4+h] = sum_i (-2 * znt[d, i]) * znt[d, 2048+i]
        for h in range(2):
            pd_scr = scrp.tile([128, RPC], BF16, tag="pd_scr")
            nc.vector.scalar_tensor_tensor(
                out=pd_scr,
                in0=znt[:, h, 0:RPC],
                scalar=-2.0,
                in1=znt[:, h, B : B + RPC],
                op0=ALU.mult,
                op1=ALU.mult,
                accum_out=tail6[:, 4 + h : 5 + h],
            )

        # ---- main matmul + exp + row-sum accumulation
        # col-group-major, with col groups (1024) aligned to the norm pair
        # groups so each group's matmuls start as soon as its own two chunks
        # are normalized.
        for cg in range(4):
            for rc in range(4):
                ps = simp.tile([128, 1024], F32, tag="ps")
                for h in range(2):
                    lhsT = znt[:, h, rc * 128 : (rc + 1) * 128]
                    for q in range(2):
                        cq = cg * 1024 + q * CW
                        nc.tensor.matmul(
                            ps[:, q * CW : (q + 1) * CW],
                            lhsT,
                            znt[:, h, cq : cq + CW],
                            start=(h == 0),
                            stop=(h == 1),
                        )
                scr = scrp.tile([128, 1024], BF16, tag="exp_scr")
                k = cg * 4 + rc
                nc.scalar.activation(
                    scr,
                    ps,
                    AF.Exp,
                    bias=zero_col,
                    scale=2.0,
                    accum_out=acc16[:, k : k + 1],
                )

        # ---- tail: rowsums, log(neg), total partial
        # acc16 col k = cg*4 + rc; rowsum4[:, rc] = sum_cg acc16[:, cg*4+rc]
        acc_v = acc16.rearrange("p (s r) -> p r s", s=4)
        nc.vector.tensor_reduce(
            out=rowsum4, in_=acc_v, axis=AXIS.X, op=ALU.add
        )
        # tail6[:, 0:4] = ln(rowsum - e^2)
        nc.scalar.activation(tail6[:, 0:4], rowsum4, AF.Ln, bias=neg_e2)
        # partial = sum over all partitions and columns of tail6
        nc.vector.tensor_reduce(out=tail1, in_=tail6, axis=AXIS.X, op=ALU.add)
        nc.gpsimd.partition_all_reduce(
            result, tail1, channels=128, reduce_op=bass_isa.ReduceOp.add
        )
        nc.sync.dma_start(out=out, in_=result[0:1, :])


_NC_CACHE = None


def _get_nc():
    global _NC_CACHE
    if _NC_CACHE is None:
        _NC_CACHE = build_nc()
    return _NC_CACHE


def make_in_maps(z_i: np.ndarray, z_j: np.ndarray):
    z = np.concatenate(
        [np.asarray(z_i, np.float32), np.asarray(z_j, np.float32)], axis=0
    )
    in_maps = []
    for c in range(NCORES):
        zr = np.roll(z, -RPC * c, axis=0)
        in_maps.append({"zt": np.ascontiguousarray(zr.T)})
    return in_maps


_EXEC_CACHE = None


def _get_exec():
    """Jitted 8-core SPMD executable, built once and reused across calls.

    Mirrors the multi-core tail of bass2jax.run_bass_via_pjrt but keeps the
    jitted function alive so repeated kernel() calls skip retrace/recompile.
    """
    global _EXEC_CACHE
    if _EXEC_CACHE is None:
        import jax
        from jax.experimental.shard_map import shard_map
        from jax.sharding import Mesh, PartitionSpec

        from concourse import bass2jax

        nc = _get_nc()
        bass2jax.install_neuronx_cc_hook()
        assert nc.dbg_addr is None
        part_name = (
            nc.partition_id_tensor.name if nc.partition_id_tensor else None
        )
        # input order: ExternalInputs, donated zeroed outputs, partition id
        in_names = ["zt", "out"] + ([part_name] if part_name else [])
        out_avals = (jax.core.ShapedArray((1, 1), np.float32),)

        def _body(*args):
            operands = list(args)
            if part_name is not None:
                operands.append(bass2jax.partition_id_tensor())
            outs = bass2jax._bass_exec_p.bind(
                *operands,
                out_avals=out_avals,
                in_names=tuple(in_names),
                out_names=("out",),
                lowering_input_output_aliases=(),
                sim_require_finite=True,
                sim_require_nnan=True,
                nc=nc,
            )
            return tuple(outs)

        devices = jax.devices()[:NCORES]
        mesh = Mesh(np.asarray(devices), ("core",))
        sharded = jax.jit(
            shard_map(
                _body,
                mesh=mesh,
                in_specs=(PartitionSpec("core"),) * 2,
                out_specs=(PartitionSpec("core"),),
                check_rep=False,
            ),
            donate_argnums=(1,),
            keep_unused=True,
        )
        _EXEC_CACHE = sharded
    return _EXEC_CACHE


def run_cores(in_maps):
    """Run the SPMD kernel; returns the 8 per-core [1,1] partials."""
    sharded = _get_exec()
    concat_zt = np.concatenate([m["zt"] for m in in_maps], axis=0)
    zeros = np.zeros((NCORES, 1), np.float32)
    (out,) = sharded(concat_zt, zeros)
    return np.asarray(out)  # [NCORES, 1]


def kernel(z_i: np.ndarray, z_j: np.ndarray) -> np.ndarray:
    partials = run_cores(make_in_maps(z_i, z_j))
    return np.float32(float(partials.sum()) / N)



# revision 5
# speedup vs baseline: 1.2325x; 1.2325x over previous
"""SimCLR NT-Xent contrastive loss on 8 Trainium2 cores (distributed).

Reference math (B=2048, D=256, T=0.5):
    zn = l2norm_rows(concat(z_i, z_j))          # [4096, 256]
    sim = zn @ zn.T / T                         # [4096, 4096]
    loss = mean_g [ log(sum_j exp(sim[g,j]) - exp(sim[g,g])) - sim[g, (g+B)%N] ]

Sharding (standard distributed SimCLR): z_i and z_j are each row-sharded
across the 8 cores — core c receives z_i[256c:256c+256] and
z_j[256c:256c+256], so the full input H2D traffic is exactly 4 MB (one fp32
copy of the data) instead of 8 replicated/rolled copies.  Each core:

  1. normalizes its own 512 rows (DVE sumsq via scalar_tensor_tensor
     accum_out, ACT exp(-0.5*ln) rsqrt, DVE scale) -> bf16
  2. transposes them on the PE (identity matmul) to znT_own [d=256, 512]
  3. AllGathers the 8 cores' znT_own blocks (128 KB each -> 1 MB) into the
     full normalized transposed z, znt [d=256, 4096]  (a column PERMUTATION
     of the reference order — harmless: row-sums and the mean are
     permutation-invariant)
  4. sim block matmuls: lhsT = znT_own col chunks, rhs = gathered znt,
     ACT exp(2*sim) with accum_out -> per-row partial rowsums
  5. positives: rows k and k+B of the reference both live on core c
     (columns k and 256+k of znT_own), and sim[g,pos] is symmetric, so
     sum_g sim[g,pos_g] over this core's rows = 4 * sum_k dot(zi_k, zj_k)
     — one local DVE scalar_tensor_tensor, no cross-core addressing.
  6. tail: log(rowsum - e^2) (sim[g,g] == 1/T for unit rows), reduce to a
     single fp32 partial; host sums the 8 partials and divides by 4096.
"""

import numpy as np

import concourse.bacc as bacc
import concourse.bass as bass
import concourse.bass_isa as bass_isa
import concourse.masks as masks
import concourse.tile as tile
from concourse import mybir

F32 = mybir.dt.float32
BF16 = mybir.dt.bfloat16
AF = mybir.ActivationFunctionType
ALU = mybir.AluOpType
AXIS = mybir.AxisListType

B = 2048
D = 256
N = 2 * B            # 4096 total rows
NCORES = 8
SPC = B // NCORES    # 256 rows of z_i (and of z_j) per core
RPC = 2 * SPC        # 512 total rows per core
E2 = float(np.exp(np.float32(2.0)))   # exp(sim[g,g]) = exp(1/T)
IN_NAMES = ("zi", "zj")   # ExternalInput order fed to the bass exec primitive
OUT_SHAPE = (1, 1)        # per-core output shape


class _Bacc(bacc.Bacc):
    """Bacc that pins the activation-table pass to the one set containing
    both Ln and Exp — the default fixpoint picks per-function sets and
    thrashes 5 table loads (~6.4us of ACT) into the schedule."""

    def insert_act_table_loads(self):
        from concourse.hw_specs import get_activation_tables
        import bass_rust as _bass_rust

        has_activation = any(
            isinstance(i, mybir.InstActivation)
            for b in self.main_func.blocks
            for i in b.instructions
        )
        if not has_activation:
            return
        keep = {
            mybir.ActivationFunctionType.Ln,
            mybir.ActivationFunctionType.Exp,
        }
        tables = [
            (k, v if k == "natural_log_exp_and_others" else v - keep)
            for k, v in get_activation_tables(self.m.arch).items()
        ]
        _bass_rust.insert_act_table_loads(self, tables)


def build_nc():
    nc = _Bacc("TRN2", target_bir_lowering=False, debug=False)
    zi = nc.dram_tensor("zi", [SPC, D], F32, kind="ExternalInput").ap()
    zj = nc.dram_tensor("zj", [SPC, D], F32, kind="ExternalInput").ap()
    out = nc.dram_tensor("out", [1, 1], F32, kind="ExternalOutput").ap()
    with tile.TileContext(nc) as tc:
        build_tile_program(tc, out, zi, zj)
    nc.compile()
    return nc


def build_tile_program(tc: tile.TileContext, out: bass.AP, zi: bass.AP, zj: bass.AP):
    nc = tc.nc
    # local row r = t*128 + p for t in 0..3: t in {0,1} from zi, {2,3} from zj
    zi_v = zi.rearrange("(t p) d -> p t d", t=2)
    zj_v = zj.rearrange("(t p) d -> p t d", t=2)

    with (
        tc.tile_pool(name="consts", bufs=1) as consts,
        tc.tile_pool(name="zp", bufs=1) as zp,
        tc.tile_pool(name="sqp", bufs=1) as sqp,
        tc.tile_pool(name="smalls", bufs=1) as smalls,
        tc.tile_pool(name="znop", bufs=1) as znop,
        tc.tile_pool(name="zntp", bufs=1) as zntp,
        tc.tile_pool(name="scrp", bufs=2) as scrp,
        tc.tile_pool(name="trps", bufs=1, space="PSUM") as trps,
        tc.tile_pool(name="simp", bufs=3, space="PSUM") as simp,
        tc.tile_pool(name="dram", bufs=1, space="DRAM") as dram,
    ):
        identity = consts.tile([128, 128], BF16, tag="identity")
        masks.make_identity(nc, identity[:, :])
        zero_col = consts.tile([128, 1], F32, tag="zero_col")
        nc.vector.memset(zero_col, 0.0)
        neg_e2 = consts.tile([128, 1], F32, tag="neg_e2")
        nc.vector.memset(neg_e2, -E2)

        # persistent tiles
        z4 = zp.tile([128, 4, D], F32)            # own rows, [p, t, d]
        sq4 = sqp.tile([128, 4, D], BF16)         # squares scratch
        zn4 = znop.tile([128, 4, D], BF16)        # normalized own rows
        znto = zntp.tile([128, 2, RPC], BF16)     # znT own block [p, h, c]
        znt = zntp.tile([128, 2, N], BF16)        # gathered znT, all 4096 cols
        ss4 = smalls.tile([128, 4], F32, tag="ss4")
        ln4 = smalls.tile([128, 4], F32, tag="ln4")
        inv4 = smalls.tile([128, 4], F32, tag="inv4")
        acc16 = smalls.tile([128, 16], F32, tag="acc16")
        tail6 = smalls.tile([128, 6], F32, tag="tail6")
        rowsum4 = smalls.tile([128, 4], F32, tag="rowsum4")
        tail1 = smalls.tile([128, 1], F32, tag="tail1")
        result = smalls.tile([128, 1], F32, tag="result")

        cc_in = dram.tile([128, 2, RPC], BF16)
        cc_out = dram.tile([NCORES, 128, 2, RPC], BF16, addr_space="Shared")

        # ---- load own rows + row sumsq (DVE accum_out over free dim)
        nc.sync.dma_start(out=z4[:, 0:2, :], in_=zi_v)
        nc.sync.dma_start(out=z4[:, 2:4, :], in_=zj_v)
        for t in range(4):
            nc.vector.scalar_tensor_tensor(
                out=sq4[:, t, :],
                in0=z4[:, t, :],
                scalar=1.0,
                in1=z4[:, t, :],
                op0=ALU.mult,
                op1=ALU.mult,
                accum_out=ss4[:, t : t + 1],
            )

        # inv = exp(-0.5 * ln(sumsq)) = 1/sqrt(sumsq)   (ACT Rsqrt is banned)
        nc.scalar.activation(ln4, ss4, AF.Ln, bias=zero_col)
        nc.scalar.activation(inv4, ln4, AF.Exp, bias=zero_col, scale=-0.5)

        # zn = z * inv  (per-partition scalar broadcast), bf16
        for t in range(4):
            nc.vector.tensor_scalar_mul(zn4[:, t, :], z4[:, t, :], inv4[:, t : t + 1])

        # ---- PE transpose own rows -> znT own block [d, c] (c = t*128 + p)
        psT = trps.tile([128, 2, RPC], BF16)
        for t in range(4):
            for h in range(2):
                nc.tensor.transpose(
                    psT[:, h, t * 128 : (t + 1) * 128],
                    zn4[:, t, h * 128 : (h + 1) * 128],
                    identity,
                )
        nc.vector.tensor_copy(znto, psT)

        # ---- AllGather the 8 blocks -> full znt (column-permuted, harmless)
        nc.sync.dma_start(out=cc_in, in_=znto)
        nc.gpsimd.collective_compute(
            "AllGather",
            ALU.bypass,
            replica_groups=[list(range(NCORES))],
            ins=[cc_in[:, :, :].opt()],
            outs=[cc_out[:, :, :, :].opt()],
        )
        for r in range(NCORES):
            nc.sync.dma_start(
                out=znt[:, :, r * RPC : (r + 1) * RPC], in_=cc_out[r]
            )

        # ---- positive-pair dots (both pair members are local):
        # sum_g sim[g, pos_g] over this core's 512 rows = 4 * sum_k zi_k.zj_k
        for h in range(2):
            pd_scr = scrp.tile([128, SPC], BF16, tag="pd_scr")
            nc.vector.scalar_tensor_tensor(
                out=pd_scr,
                in0=znto[:, h, 0:SPC],
                scalar=-4.0,
                in1=znto[:, h, SPC:RPC],
                op0=ALU.mult,
                op1=ALU.mult,
                accum_out=tail6[:, 4 + h : 5 + h],
            )

        # ---- main matmul + exp + row-sum accumulation
        for cg in range(4):
            for rc in range(4):
                ps = simp.tile([128, 1024], F32, tag="ps")
                for h in range(2):
                    lhsT = znto[:, h, rc * 128 : (rc + 1) * 128]
                    for q in range(2):
                        cq = cg * 1024 + q * 512
                        nc.tensor.matmul(
                            ps[:, q * 512 : (q + 1) * 512],
                            lhsT,
                            znt[:, h, cq : cq + 512],
                            start=(h == 0),
                            stop=(h == 1),
                        )
                scr = scrp.tile([128, 1024], BF16, tag="exp_scr")
                k = cg * 4 + rc
                nc.scalar.activation(
                    scr,
                    ps,
                    AF.Exp,
                    bias=zero_col,
                    scale=2.0,
                    accum_out=acc16[:, k : k + 1],
                )

        # ---- tail: rowsums, log(neg), total partial
        acc_v = acc16.rearrange("p (s r) -> p r s", s=4)
        nc.vector.tensor_reduce(out=rowsum4, in_=acc_v, axis=AXIS.X, op=ALU.add)
        nc.scalar.activation(tail6[:, 0:4], rowsum4, AF.Ln, bias=neg_e2)
        nc.vector.tensor_reduce(out=tail1, in_=tail6, axis=AXIS.X, op=ALU.add)
        nc.gpsimd.partition_all_reduce(
            result, tail1, channels=128, reduce_op=bass_isa.ReduceOp.add
        )
        nc.sync.dma_start(out=out, in_=result[0:1, :])


_NC_CACHE = None


def _get_nc():
    global _NC_CACHE
    if _NC_CACHE is None:
        _NC_CACHE = build_nc()
    return _NC_CACHE


def make_in_maps(z_i: np.ndarray, z_j: np.ndarray):
    z_i = np.asarray(z_i, np.float32)
    z_j = np.asarray(z_j, np.float32)
    return [
        {"zi": z_i[c * SPC : (c + 1) * SPC], "zj": z_j[c * SPC : (c + 1) * SPC]}
        for c in range(NCORES)
    ]


_EXEC_CACHE = None


def _get_exec():
    """Jitted 8-core SPMD executable, built once and reused across calls."""
    global _EXEC_CACHE
    if _EXEC_CACHE is None:
        import jax
        from jax.experimental.shard_map import shard_map
        from jax.sharding import Mesh, PartitionSpec

        from concourse import bass2jax

        nc = _get_nc()
        bass2jax.install_neuronx_cc_hook()
        assert nc.dbg_addr is None
        part_name = (
            nc.partition_id_tensor.name if nc.partition_id_tensor else None
        )
        # input order: ExternalInputs, donated zeroed outputs, partition id
        in_names = list(IN_NAMES) + ["out"] + ([part_name] if part_name else [])
        out_avals = (jax.core.ShapedArray(OUT_SHAPE, np.float32),)

        def _body(*args):
            operands = list(args)
            if part_name is not None:
                operands.append(bass2jax.partition_id_tensor())
            outs = bass2jax._bass_exec_p.bind(
                *operands,
                out_avals=out_avals,
                in_names=tuple(in_names),
                out_names=("out",),
                lowering_input_output_aliases=(),
                sim_require_finite=True,
                sim_require_nnan=True,
                nc=nc,
            )
            return tuple(outs)

        devices = jax.devices()[:NCORES]
        mesh = Mesh(np.asarray(devices), ("core",))
        n_in = len(IN_NAMES)
        sharded = jax.jit(
            shard_map(
                _body,
                mesh=mesh,
                in_specs=(PartitionSpec("core"),) * (n_in + 1),
                out_specs=(PartitionSpec("core"),),
                check_rep=False,
            ),
            donate_argnums=(n_in,),
            keep_unused=True,
        )
        _EXEC_CACHE = sharded
    return _EXEC_CACHE


def kernel(z_i: np.ndarray, z_j: np.ndarray) -> np.ndarray:
    """Full inputs in, full output out; shards rows across the 8 cores."""
    sharded = _get_exec()
    zeros = np.zeros((NCORES * OUT_SHAPE[0], OUT_SHAPE[1]), np.float32)
    (partials,) = sharded(
        np.asarray(z_i, np.float32), np.asarray(z_j, np.float32), zeros
    )
    return np.float32(float(np.asarray(partials).sum()) / N)
